# revision 12
# baseline (speedup 1.0000x reference)
"""Trainium2 Bass kernel for ChannelHyperedgeInteraction.

Computation (per batch b):
    E  = masked-mean-pool of x over L              [C, d]
    qkv = E @ Wqkv + bqkv ; q,k,v                  [C, d] each
    S  = (q k^T / sqrt(d)) * (0.5 + 0.5*overlap)   [C, C]
    A  = softmax(S, -1) ; out = A v @ Wo + bo
    E  = LN(E + out) ; h = gelu(E W1 + b1) W2 + b2 ; return LN(E + h)

Sharding: data-parallel over B across the 8 NeuronCores (2 batches/core).
Weights are replicated. Each core computes its own y[b_local] slice; the
host concatenates.

Schedule (the dominant cost is streaming x, 134 MB/core, at the DMA
roofline ~400 GB/s):
 - x tiles alternate between the SP and ACT HWDGE rings (per-ring cap is
   ~210 GB/s; both must stay busy).  Weights/biases/broadcasts/outputs
   ride the Pool ring exclusively so they never head-of-line block an
   x tile (the previous version injected weight DMAs into the SP ring
   mid-stream, which serialized both rings for ~90 us).
 - Both batches' mask prep happens up front; transformer(0) is split
   into stages injected between pooling(1) blocks, so the PE keeps
   consuming x tiles (freeing DMA ring buffers) while batch 0's
   attention/FFN runs.  Only transformer(1) is a serial tail.
 - The masked pooling is done on the TensorEngine as
   E += diag(mask_scaled[:, l]) @ x[b, :, l, :] accumulated in PSUM;
   fp32r keeps the moving operand at 1 column/cycle.
 - HAM (the PE activity clock monitor) halves the core clock when PE
   looks idle; during the DMA-bound stream that is harmless (PE keeps
   up with the rings even at half clock), but the serial tail would run
   2x slow.  Throwaway "warm" matmuls pad the PE's idle gaps through
   the last pooling blocks and the tail DVE/ACT stretches to hold the
   clock at 2.4 GHz.  They accumulate into an unused strip of the gate
   PSUM bank.
 - The walrus codegen accepts at most ONE sync wait per ISA
   instruction: HWDGE DMA completions are pinned one semaphore lane PER
   RING (SP=0, ACT=1, Pool=2; rings complete FIFO so cumulative waits
   are safe), 1x1 "gate" matmuls make PE observe DMA semaphores on
   cheap instructions, and a post-pass spills any remaining multi-waits
   onto same-engine NoOps.
"""

import math
from contextlib import ExitStack

import numpy as np

import concourse.bass as bass
import concourse.mybir as mybir
import concourse.tile as tile
from concourse.bass_utils import run_bass_kernel_spmd
from concourse.masks import make_identity

F32 = mybir.dt.float32
F32R = mybir.dt.float32r
AX = mybir.AxisListType
OP = mybir.AluOpType
ACTF = mybir.ActivationFunctionType

B, C, L, D = 16, 128, 256, 512
N_CORES = 8
B_LOCAL = B // N_CORES  # 2
P = 128
LC = L // P   # 2 l-chunks
DC = D // P   # 4 d-chunks
D2C = (2 * D) // P  # 8 chunks of the FFN hidden dim
NLB = 4  # l-positions per x DMA (1 MB transfers)
NBLK = L // NLB

WEIGHT_NAMES = ("Wqkv", "bqkv", "Wo", "bo", "W1", "b1", "W2", "b2",
                "g1", "be1", "g2", "be2")


def _build_kernel_body(ctx: ExitStack, tc: "tile.TileContext", io: dict):
    nc = tc.nc
    x, mask, y = io["x"], io["mask"], io["y"]

    singles = ctx.enter_context(tc.tile_pool(name="singles", bufs=1))
    xpool = ctx.enter_context(tc.tile_pool(name="xpool", bufs=8))
    work2 = ctx.enter_context(tc.tile_pool(name="work2", bufs=2))
    work1 = ctx.enter_context(tc.tile_pool(name="work1", bufs=1))
    psE = ctx.enter_context(tc.tile_pool(name="psE", bufs=1, space="PSUM"))
    psT = ctx.enter_context(tc.tile_pool(name="psT", bufs=2, space="PSUM"))
    psMM = ctx.enter_context(tc.tile_pool(name="psMM", bufs=4, space="PSUM"))
    psD = ctx.enter_context(tc.tile_pool(name="psD", bufs=1, space="PSUM"))

    ones_row = singles.tile([1, P], F32)
    nc.vector.memset(ones_row, 1.0)
    eps_t = singles.tile([P, 1], F32)
    nc.vector.memset(eps_t, 1e-5)
    # scratch operand for HAM warm-keeper matmuls (content irrelevant)
    warm_sb = singles.tile([P, P], F32)
    nc.vector.memset(warm_sb, 1.0)
    # fp32 identity: the ONLY gpsimd-built constant, created first so the
    # PE can start consuming x within ~2us of kernel start (if the PE ever
    # falls behind the rings, the dma_start issues begin blocking on PE
    # progress and the stream degenerates into a few-tiles-in-flight limit
    # cycle at ~70% throughput).  The warmup transpose makes PE observe the
    # gpsimd semaphore at its latest tick, so no later PE instruction needs
    # a Pool wait.
    ident = singles.tile([P, P], F32)
    make_identity(nc, ident)
    wu_ps = psT.tile([P, P], F32, tag="pst")
    nc.tensor.transpose(wu_ps, ident, ident)
    # identity blocks (template for the pooling diag weights): replicated
    # from `ident` on the DVE, which is much faster than 4 gpsimd
    # affine-selects.
    identN = singles.tile([P, NLB, P], F32)
    for i in range(NLB):
        nc.vector.tensor_copy(identN[:, i], ident)

    # --- masks: both batches' DMAs first in the SP ring ---------------------
    mb_t = {}
    for b in range(B_LOCAL):
        t = work1.tile([P, L], F32, tag=f"mb{b}", name=f"mb{b}")
        nc.sync.dma_start(out=t, in_=mask[b])
        mb_t[b] = t

    # --- weights: all on the Pool ring, issued up front ---------------------
    # (their completions are observed via gate matmuls after pooling(0))
    w = {}

    def big_w(key, src_name, nch, width):
        t = singles.tile([P, nch, width], F32R, name=f"{key}_sb")
        nc.gpsimd.dma_start(
            out=t,
            in_=io[src_name][:].bitcast(F32R).rearrange(
                "(j p) n -> p j n", p=P))
        w[key] = t

    big_w("wqkv", "Wqkv", DC, 3 * D)
    big_w("w1", "W1", DC, 2 * D)
    big_w("w2", "W2", D2C, D)
    big_w("wo", "Wo", DC, D)

    for nm, width in (("bqkv", 3 * D), ("bo", D), ("b1", 2 * D), ("b2", D)):
        t = singles.tile([1, width], F32, name=f"row_{nm}")
        nc.gpsimd.dma_start(out=t, in_=io[nm][None, :])
        w[nm] = t

    for nm in ("g1", "be1", "g2", "be2"):
        t = singles.tile([P, D], F32, name=f"bc_{nm}")
        nc.gpsimd.dma_start(out=t, in_=io[nm][None, :].to_broadcast((P, D)))
        w[nm] = t

    # One kernel-long accumulation group of 1x1 "gate" matmuls, used to make
    # PE observe the (8-lane round-robin) weight DMA completion semaphores
    # on cheap instructions before the first weight use.  The same PSUM bank
    # hosts the warm-keeper accumulator in a disjoint column strip.
    N_GATES = 8
    dw_ps = psD.tile([P, 2 * P], F32, tag="dw", name="dw_ps")
    _gate = {"i": 0}

    def gate_mm(el):
        nc.tensor.matmul(dw_ps[0:1, 0:1], lhsT=el, rhs=el,
                         start=(_gate["i"] == 0),
                         stop=(_gate["i"] == N_GATES - 1))
        _gate["i"] += 1

    def observe_weight_dmas():
        for nm in ("wqkv", "w1", "w2", "wo"):
            gate_mm(w[nm][0:1, 0, 0:1].bitcast(F32))
        for nm in ("bqkv", "bo", "b1", "b2"):
            gate_mm(w[nm][0:1, 0:1])

    # HAM warm-keeper: N=128 fp32 matmuls (4-pass, ~213ns each at full
    # clock), sprinkled through PE-idle stretches so the HAM activity
    # monitor keeps the clock at 2.4 GHz (PE transposes and waits don't
    # register as "busy", so K drops to 4/8 otherwise).  Each tick READS
    # the caller-supplied `anchor` tile: the Tile scheduler topologically
    # reorders each engine's queue, so a dependency-free matmul would be
    # hoisted to the very front of the PE stream instead of staying in
    # its block (values are irrelevant — the strip is never read).
    _warm = {"open": False, "n": 0}

    def warm_tick(k, anchor):
        a = anchor if anchor.dtype == F32 else anchor.bitcast(F32)
        for _ in range(k):
            nc.tensor.matmul(dw_ps[:, P:2 * P], lhsT=a, rhs=a,
                             start=(not _warm["open"]), stop=False)
            _warm["open"] = True
            _warm["n"] += 1

    def warm_finish():
        if _warm["open"]:
            nc.tensor.matmul(dw_ps[:, P:2 * P], lhsT=warm_sb, rhs=warm_sb,
                             start=False, stop=True)
            _warm["open"] = False

    def bias_mm(psum_ap, bias_row_ap):
        """Final accumulation-group matmul adding a [1, N] bias row to all
        output rows: psum += ones[K=1, M=P].T @ bias[K=1, N]."""
        nc.tensor.matmul(psum_ap, lhsT=ones_row,
                         rhs=bias_row_ap, start=False, stop=True)

    def transpose_chunks(src, nch, tag, dtype=F32):
        """[P, nch*P] SBUF -> [P, nch, P] SBUF holding src^T chunks."""
        dst = work1.tile([P, nch, P], dtype, tag=tag)
        for j in range(nch):
            ps = psT.tile([P, P], F32, tag="pst")
            nc.tensor.transpose(ps, src[:, j * P:(j + 1) * P], ident)
            nc.vector.tensor_copy(dst[:, j], ps)
        return dst

    def layernorm(src, g_b, be_b, tag):
        stats = work1.tile([P, 6], F32, tag=tag + "_st")
        nc.vector.bn_stats(out=stats, in_=src)
        mv = work1.tile([P, 2], F32, tag=tag + "_mv")
        nc.vector.bn_aggr(out=mv, in_=stats)
        rstd = work1.tile([P, 1], F32, tag=tag + "_rs")
        nc.scalar.activation(rstd, mv[:, 1:2], ACTF.Sqrt, bias=eps_t)
        nc.vector.reciprocal(rstd, rstd)
        out_t = work1.tile([P, D], F32, tag=tag)
        nc.vector.tensor_scalar(out_t, src, scalar1=mv[:, 0:1], scalar2=rstd,
                                op0=OP.subtract, op1=OP.mult)
        nc.vector.tensor_mul(out_t, out_t, g_b)
        nc.vector.tensor_add(out_t, out_t, be_b)
        return out_t

    def mask_prep(b):
        """Returns (ms_t [P,L] row-normalized mask, factor [P,P])."""
        mb = mb_t[b]
        total = work1.tile([P, 1], F32, tag="total")
        nc.vector.reduce_sum(total, mb, axis=AX.X, op=OP.add)
        rden = work1.tile([P, 1], F32, tag="rden")
        nc.vector.tensor_scalar_max(rden, total, 1.0)
        nc.vector.reciprocal(rden, rden)
        ms_t = work1.tile([P, L], F32, tag=f"ms{b}", name=f"ms{b}")
        nc.vector.tensor_scalar_mul(ms_t, mb, rden)

        mT = transpose_chunks(mb, LC, "mT")      # raw mask^T  [l, c]

        # joint[c,e] = sum_l m[c,l] m[e,l]
        joint_ps = psMM.tile([P, P], F32, tag="mm")
        for ch in range(LC):
            nc.tensor.matmul(joint_ps, lhsT=mT[:, ch], rhs=mT[:, ch],
                             start=(ch == 0), stop=(ch == LC - 1))
        # broadcast total^T along rows
        totT_ps = psT.tile([1, P], F32, tag="pst")
        nc.tensor.transpose(totT_ps, total, ident)
        tot_row = work1.tile([1, P], F32, tag="totrow")
        nc.vector.tensor_copy(tot_row, totT_ps)
        totb_ps = psT.tile([P, P], F32, tag="pst")
        nc.tensor.matmul(totb_ps, lhsT=ones_row, rhs=tot_row,
                         start=True, stop=True)
        # factor = (0.5 + joint / max(total[c]+total[e], 1)) / sqrt(D)
        factor = work1.tile([P, P], F32, tag=f"factor{b}", name=f"factor{b}")
        nc.vector.tensor_scalar_add(factor, totb_ps, total)
        nc.vector.tensor_scalar_max(factor, factor, 1.0)
        nc.vector.reciprocal(factor, factor)
        nc.vector.tensor_mul(factor, factor, joint_ps)
        nc.vector.tensor_scalar(factor, factor, 0.5, 1.0 / math.sqrt(D),
                                op0=OP.add, op1=OP.mult)
        return ms_t, factor

    def pooling(b, ms_t, inject=None, warm_k=0):
        """Masked-mean pooling -> E_sb [P(c), D].

        E += diag(ms[:, l]) @ x[b, :, l, :], accumulated over l in PSUM.
        inject: optional {block_idx: [fn, ...]} extra issue hooks (used to
        interleave the previous batch's transformer stages into the
        stream).
        """
        psum_E = psE.tile([P, D], F32, tag="psE")
        for ib, l0 in enumerate(range(0, L, NLB)):
            xt = xpool.tile([P, NLB, D], F32R, tag="xt")
            eng = nc.sync if ib % 2 == 0 else nc.scalar
            eng.dma_start(out=xt, in_=x[b, :, l0:l0 + NLB, :].bitcast(F32R))
            if inject and ib in inject:
                for fn in inject[ib]:
                    fn()
            diag = work2.tile([P, NLB, P], F32R, tag="diag")
            nc.vector.tensor_tensor(
                diag, identN,
                ms_t[:, l0:l0 + NLB, None].to_broadcast((P, NLB, P)),
                OP.mult)
            for i in range(NLB):
                nc.tensor.matmul(
                    psum_E,
                    lhsT=diag[:, i],
                    rhs=xt[:, i],
                    start=(l0 == 0 and i == 0),
                    stop=(l0 == L - NLB and i == NLB - 1),
                )
            if warm_k:
                warm_tick(warm_k, xt[:, 0, 0:P])
        E_sb = work2.tile([P, D], F32, tag="E")
        nc.vector.tensor_copy(E_sb, psum_E)
        return E_sb

    def transformer_stages(b, E_box, factor, warm=False):
        """Returns a list of stage closures computing y[b] from E_box[0].

        warm=True pads the PE-idle gaps after DVE/ACT-heavy stages with
        warm-keeper matmuls (used for the serial tail batch).
        """
        st = {}

        def wt(k, anchor):
            if warm:
                warm_tick(k, anchor)

        def s_et():
            st["ET"] = transpose_chunks(E_box[0], DC, "ET", F32R)

        def mk_qkv(i, name, dtype):
            def go():
                ps = psMM.tile([P, D], F32, tag="mm", name=f"qkv_ps{i}")
                for j in range(DC):
                    nc.tensor.matmul(ps, lhsT=st["ET"][:, j],
                                     rhs=w["wqkv"][:, j, i * D:(i + 1) * D],
                                     start=(j == 0), stop=False)
                bias_mm(ps, w["bqkv"][:, i * D:(i + 1) * D])
                t = work1.tile([P, D], dtype, tag=name, name=name)
                nc.vector.tensor_copy(t, ps)
                st[name] = t
            return go

        def s_qkT():
            st["qT"] = transpose_chunks(st["q"], DC, "qT")
            st["kT"] = transpose_chunks(st["k"], DC, "kT")

        def s_S():
            S_ps = psMM.tile([P, P], F32, tag="mm")
            for j in range(DC):
                nc.tensor.matmul(S_ps, lhsT=st["qT"][:, j],
                                 rhs=st["kT"][:, j],
                                 start=(j == 0), stop=(j == DC - 1))
            Sb = work1.tile([P, P], F32, tag="Sb")
            nc.vector.tensor_mul(Sb, S_ps, factor)
            nmax = work1.tile([P, 1], F32, tag="nmax")
            nc.vector.reduce_max(nmax, Sb, axis=AX.X, negate=True)
            st["Sb"], st["nmax"] = Sb, nmax
            wt(3, Sb)

        def s_soft():
            Pexp = work1.tile([P, P], F32, tag="Pexp")
            sumexp = work1.tile([P, 1], F32, tag="sumexp")
            nc.scalar.activation(Pexp, st["Sb"], ACTF.Exp, bias=st["nmax"],
                                 scale=1.0, accum_out=sumexp)
            rinv = work1.tile([P, 1], F32, tag="rinv")
            nc.vector.reciprocal(rinv, sumexp)
            # normalize rows of exp(S) so attn = A @ v directly
            nc.vector.tensor_scalar_mul(Pexp, Pexp, rinv)
            st["Pexp"] = Pexp
            wt(4, Pexp)

        def s_PT():
            PT_ps = psT.tile([P, P], F32, tag="pst")
            nc.tensor.transpose(PT_ps, st["Pexp"], ident)
            PT_sb = work1.tile([P, P], F32R, tag="PT")
            nc.vector.tensor_copy(PT_sb, PT_ps)
            st["PT"] = PT_sb
            wt(2, PT_sb)

        def s_attn():
            attn_ps = psMM.tile([P, D], F32, tag="mm")
            nc.tensor.matmul(attn_ps, lhsT=st["PT"], rhs=st["v"],
                             start=True, stop=True)
            attn_sb = work1.tile([P, D], F32, tag="attnsb")
            nc.vector.tensor_copy(attn_sb, attn_ps)
            st["attn"] = attn_sb
            wt(2, attn_sb[:, 0:P])

        def s_attnT():
            st["attnT"] = transpose_chunks(st["attn"], DC, "attnT", F32R)

        def s_o_ln1():
            o_ps = psMM.tile([P, D], F32, tag="mm")
            for j in range(DC):
                nc.tensor.matmul(o_ps, lhsT=st["attnT"][:, j],
                                 rhs=w["wo"][:, j],
                                 start=(j == 0), stop=False)
            bias_mm(o_ps, w["bo"])
            res1 = work1.tile([P, D], F32, tag="res1")
            nc.vector.tensor_add(res1, o_ps, E_box[0])
            st["E1"] = layernorm(res1, w["g1"], w["be1"], "E1")
            wt(5, st["E1"][:, 0:P])

        def s_E1T():
            st["E1T"] = transpose_chunks(st["E1"], DC, "E1T", F32R)

        def mk_h(i):
            def go():
                ps = psMM.tile([P, D], F32, tag="mm", name=f"h_ps{i}")
                for j in range(DC):
                    nc.tensor.matmul(ps, lhsT=st["E1T"][:, j],
                                     rhs=w["w1"][:, j, i * D:(i + 1) * D],
                                     start=(j == 0), stop=False)
                bias_mm(ps, w["b1"][:, i * D:(i + 1) * D])
                st[f"h{i}"] = ps
            return go

        def mk_gelu(i):
            def go():
                # reuse a dead [P, D] slot (res1 after LN1 / attnsb after
                # its transposes) for the gelu input copy
                hx = work1.tile([P, D], F32, tag="res1" if i == 0 else "attnsb",
                                name=f"hx{i}")
                nc.vector.tensor_copy(hx, st[f"h{i}"])
                g = work1.tile([P, D], F32, tag=f"g{i}", name=f"g{i}")
                nc.vector.tensor_mul(g, hx, hx)
                nc.vector.tensor_scalar(g, g, 0.044715, 1.0,
                                        op0=OP.mult, op1=OP.add)
                nc.vector.tensor_mul(g, g, hx)
                nc.scalar.activation(g, g, ACTF.Tanh,
                                     scale=math.sqrt(2.0 / math.pi))
                nc.vector.tensor_scalar(g, g, 1.0, 0.5,
                                        op0=OP.add, op1=OP.mult)
                nc.vector.tensor_mul(g, g, hx)
                st[f"g{i}"] = g
                wt(4, g[:, 0:P])
            return go

        def s_hT():
            dst = work1.tile([P, D2C, P], F32R, tag="hT")
            for half in range(2):
                src = st[f"g{half}"]
                for j in range(DC):
                    ps = psT.tile([P, P], F32, tag="pst")
                    nc.tensor.transpose(ps, src[:, j * P:(j + 1) * P], ident)
                    nc.vector.tensor_copy(dst[:, half * DC + j], ps)
            st["hT"] = dst

        def s_out():
            o2_ps = psMM.tile([P, D], F32, tag="mm")
            for jj in range(D2C):
                nc.tensor.matmul(o2_ps, lhsT=st["hT"][:, jj],
                                 rhs=w["w2"][:, jj],
                                 start=(jj == 0), stop=False)
            bias_mm(o2_ps, w["b2"])
            res2 = work1.tile([P, D], F32, tag="res2")
            nc.vector.tensor_add(res2, o2_ps, st["E1"])
            y_sb = layernorm(res2, w["g2"], w["be2"], "yln")
            nc.gpsimd.dma_start(out=y[b], in_=y_sb)

        return [s_et,
                mk_qkv(0, "q", F32), mk_qkv(1, "k", F32), mk_qkv(2, "v", F32R),
                s_qkT, s_S, s_soft, s_PT, s_attn, s_attnT, s_o_ln1,
                s_E1T, mk_h(0), mk_h(1), mk_gelu(0), mk_gelu(1),
                s_hT, s_out]

    # ---- main schedule ----------------------------------------------------
    ms0, factor0 = mask_prep(0)
    ms1, factor1 = mask_prep(1)

    E_box = [None]
    # warm_k=1: one ~213ns throwaway matmul per 2.56us block keeps the HAM
    # activity monitor from halving the clock mid-stream (at half clock the
    # PE's margin over the DMA delivery rate is too thin to stay ahead).
    E_box[0] = pooling(0, ms0, warm_k=1)
    observe_weight_dmas()

    # transformer(0) stages ride between pooling(1) blocks so the x stream
    # never stalls on a busy PE.
    stages0 = transformer_stages(0, E_box, factor0)
    inject = {1 + 2 * i: [fn] for i, fn in enumerate(stages0)}
    E1_sb = pooling(1, ms1, inject=inject, warm_k=1)

    E_box[0] = E1_sb
    for fn in transformer_stages(1, E_box, factor1, warm=True):
        fn()
    warm_finish()


def build_module() -> bass.Bass:
    # HWDGE DMA completions: one semaphore lane PER PHYSICAL RING (SP ring
    # -> lane 0, ACT ring -> lane 1, Pool ring -> lane 2). DMAs issued from
    # one engine's ring complete FIFO, so cumulative waits on that ring's
    # lane are safe, and every DMA-dependent instruction needs at most one
    # DMA semaphore wait per ring (the walrus codegen accepts only ONE sync
    # wait per ISA instruction; the NoOp spill pass below handles any
    # leftovers).
    import concourse.tile_sem_assignment as _tsa
    _tsa.NUM_HWDGE_SEMS = 3
    if not getattr(_tsa.TileClockTick, "_ring_lane_patch", False):
        _orig_assign_tick = _tsa.TileClockTick._assign_tick

        def _assign_tick_ring_lane(self, inst):
            if isinstance(inst, _tsa.DMAInst):
                if inst.engine == mybir.EngineType.Activation:
                    self.next_hw_dma_idx = 1
                elif inst.engine == mybir.EngineType.SP:
                    self.next_hw_dma_idx = 0
                elif inst.engine == mybir.EngineType.Pool:
                    self.next_hw_dma_idx = 2
            return _orig_assign_tick(self, inst)

        _tsa.TileClockTick._assign_tick = _assign_tick_ring_lane
        _tsa.TileClockTick._ring_lane_patch = True

    nc = bass.Bass()
    io = {}
    io["x"] = nc.declare_dram_parameter("x", [B_LOCAL, C, L, D], F32,
                                        isOutput=False)
    io["mask"] = nc.declare_dram_parameter("mask", [B_LOCAL, C, L], F32,
                                           isOutput=False)
    shapes = {
        "Wqkv": [D, 3 * D], "bqkv": [3 * D], "Wo": [D, D], "bo": [D],
        "W1": [D, 2 * D], "b1": [2 * D], "W2": [2 * D, D], "b2": [D],
        "g1": [D], "be1": [D], "g2": [D], "be2": [D],
    }
    for nm in WEIGHT_NAMES:
        io[nm] = nc.declare_dram_parameter(nm, shapes[nm], F32, isOutput=False)
    io["y"] = nc.declare_dram_parameter("y", [B_LOCAL, C, D], F32,
                                        isOutput=True)

    with tile.TileContext(nc) as tc:
        with ExitStack() as ctx:
            _build_kernel_body(ctx, tc, io)
    _split_multi_waits(nc)
    return nc


def _split_multi_waits(nc: bass.Bass) -> int:
    """The walrus codegen in this toolchain accepts at most ONE sync-wait
    command per ISA instruction. Tile's semaphore assignment can attach
    several. Spill all but the last wait of each instruction onto NoOp
    instructions (same engine, inserted just before it), each carrying a
    single wait — execution-equivalent since the engine stream is in-order.
    """
    import bass_rust as _br
    fn = nc.m.functions[0]
    n_spilled = 0
    for blk in fn.blocks:
        out = []
        changed = False
        for inst in blk.instructions:
            si = inst.sync_info
            if si is not None and len(si.on_wait) > 1:
                waits = list(si.on_wait)
                for wv in waits[:-1]:
                    n_spilled += 1
                    nop = mybir.InstNoOp(
                        name=f"I-wspill-{n_spilled}",
                        engine=inst.engine,
                        sync_info=_br.SyncInfo(on_wait=[wv], on_update=[]),
                        bass_nofuse=True,
                    )
                    nc.register_instruction(nop)
                    out.append(nop)
                inst.sync_info = _br.SyncInfo(
                    on_wait=[waits[-1]], on_update=list(si.on_update))
                changed = True
            out.append(inst)
        if changed:
            blk.instructions = out
    return n_spilled


_NC_CACHE = None


def _get_module():
    global _NC_CACHE
    if _NC_CACHE is None:
        _NC_CACHE = build_module()
    return _NC_CACHE


def kernel(**inputs) -> np.ndarray:
    arrs = {k: np.ascontiguousarray(np.asarray(v, dtype=np.float32))
            for k, v in inputs.items()}
    nc = _get_module()
    in_maps = []
    for i in range(N_CORES):
        m = {
            "x": arrs["x"][i * B_LOCAL:(i + 1) * B_LOCAL],
            "mask": arrs["mask"][i * B_LOCAL:(i + 1) * B_LOCAL],
        }
        for nm in WEIGHT_NAMES:
            m[nm] = arrs[nm]
        in_maps.append(m)
    res = run_bass_kernel_spmd(nc, in_maps, list(range(N_CORES)))
    return np.concatenate([r["y"] for r in res.results], axis=0)


if __name__ == "__main__":
    build_module()
    print("module built OK")


# revision 13
# speedup vs baseline: 1.1586x; 1.1586x over previous
"""Trainium2 Bass kernel for ChannelHyperedgeInteraction.

Computation (per batch b):
    E  = masked-mean-pool of x over L              [C, d]
    qkv = E @ Wqkv + bqkv ; q,k,v                  [C, d] each
    S  = (q k^T / sqrt(d)) * (0.5 + 0.5*overlap)   [C, C]
    A  = softmax(S, -1) ; out = A v @ Wo + bo
    E  = LN(E + out) ; h = gelu(E W1 + b1) W2 + b2 ; return LN(E + h)

Sharding: data-parallel over B across the 8 NeuronCores (2 batches/core).
Weights are replicated. Each core computes its own y[b_local] slice; the
host concatenates.

Schedule (the dominant cost is streaming x, 134 MB/core, at the DMA
roofline ~400 GB/s):
 - x tiles alternate between the SP and ACT HWDGE rings (per-ring cap is
   ~210 GB/s; both must stay busy).  Weights/biases/broadcasts/outputs
   ride the Pool ring exclusively so they never head-of-line block an
   x tile (the previous version injected weight DMAs into the SP ring
   mid-stream, which serialized both rings for ~90 us).
 - Both batches' mask prep happens up front; transformer(0) is split
   into stages injected between pooling(1) blocks, so the PE keeps
   consuming x tiles (freeing DMA ring buffers) while batch 0's
   attention/FFN runs.  Only transformer(1) is a serial tail.
 - The masked pooling is done on the TensorEngine as
   E += diag(mask_scaled[:, l]) @ x[b, :, l, :] accumulated in PSUM;
   fp32r keeps the moving operand at 1 column/cycle.
 - HAM (the PE activity clock monitor) halves the core clock when PE
   looks idle; during the DMA-bound stream that is harmless (PE keeps
   up with the rings even at half clock), but the serial tail would run
   2x slow.  Throwaway "warm" matmuls pad the PE's idle gaps through
   the last pooling blocks and the tail DVE/ACT stretches to hold the
   clock at 2.4 GHz.  They accumulate into an unused strip of the gate
   PSUM bank.
 - The walrus codegen accepts at most ONE sync wait per ISA
   instruction: HWDGE DMA completions are pinned one semaphore lane PER
   RING (SP=0, ACT=1, Pool=2; rings complete FIFO so cumulative waits
   are safe), 1x1 "gate" matmuls make PE observe DMA semaphores on
   cheap instructions, and a post-pass spills any remaining multi-waits
   onto same-engine NoOps.
"""

import math
from contextlib import ExitStack

import numpy as np

import concourse.bass as bass
import concourse.mybir as mybir
import concourse.tile as tile
from concourse.bass_utils import run_bass_kernel_spmd
from concourse.masks import make_identity

F32 = mybir.dt.float32
F32R = mybir.dt.float32r
AX = mybir.AxisListType
OP = mybir.AluOpType
ACTF = mybir.ActivationFunctionType

B, C, L, D = 16, 128, 256, 512
N_CORES = 8
B_LOCAL = B // N_CORES  # 2
P = 128
LC = L // P   # 2 l-chunks
DC = D // P   # 4 d-chunks
D2C = (2 * D) // P  # 8 chunks of the FFN hidden dim
NLB = 4  # l-positions per x DMA (1 MB transfers)
NBLK = L // NLB

WEIGHT_NAMES = ("Wqkv", "bqkv", "Wo", "bo", "W1", "b1", "W2", "b2",
                "g1", "be1", "g2", "be2")


def _build_kernel_body(ctx: ExitStack, tc: "tile.TileContext", io: dict):
    nc = tc.nc
    x, mask, y = io["x"], io["mask"], io["y"]

    singles = ctx.enter_context(tc.tile_pool(name="singles", bufs=1))
    xpool = ctx.enter_context(tc.tile_pool(name="xpool", bufs=8))
    work2 = ctx.enter_context(tc.tile_pool(name="work2", bufs=2))
    work1 = ctx.enter_context(tc.tile_pool(name="work1", bufs=1))
    psE = ctx.enter_context(tc.tile_pool(name="psE", bufs=1, space="PSUM"))
    psT = ctx.enter_context(tc.tile_pool(name="psT", bufs=2, space="PSUM"))
    psMM = ctx.enter_context(tc.tile_pool(name="psMM", bufs=4, space="PSUM"))
    psD = ctx.enter_context(tc.tile_pool(name="psD", bufs=1, space="PSUM"))

    ones_row = singles.tile([1, P], F32)
    nc.vector.memset(ones_row, 1.0)
    eps_t = singles.tile([P, 1], F32)
    nc.vector.memset(eps_t, 1e-5)
    # scratch operand for HAM warm-keeper matmuls (content irrelevant)
    warm_sb = singles.tile([P, P], F32)
    nc.vector.memset(warm_sb, 1.0)
    # fp32 identity: the ONLY gpsimd-built constant, created first so the
    # PE can start consuming x within ~2us of kernel start (if the PE ever
    # falls behind the rings, the dma_start issues begin blocking on PE
    # progress and the stream degenerates into a few-tiles-in-flight limit
    # cycle at ~70% throughput).  The warmup transpose makes PE observe the
    # gpsimd semaphore at its latest tick, so no later PE instruction needs
    # a Pool wait.
    ident = singles.tile([P, P], F32)
    make_identity(nc, ident)
    wu_ps = psT.tile([P, P], F32, tag="pst")
    nc.tensor.transpose(wu_ps, ident, ident)
    # identity blocks (template for the pooling diag weights): replicated
    # from `ident` on the DVE, which is much faster than 4 gpsimd
    # affine-selects.
    identN = singles.tile([P, NLB, P], F32)
    for i in range(NLB):
        nc.vector.tensor_copy(identN[:, i], ident)

    # --- masks: both batches' DMAs first in the SP ring ---------------------
    mb_t = {}
    for b in range(B_LOCAL):
        t = work1.tile([P, L], F32, tag=f"mb{b}", name=f"mb{b}")
        nc.sync.dma_start(out=t, in_=mask[b])
        mb_t[b] = t

    # --- weights: all on the Pool ring, issued up front ---------------------
    # (their completions are observed via gate matmuls after pooling(0))
    w = {}

    def big_w(key, src_name, nch, width):
        t = singles.tile([P, nch, width], F32R, name=f"{key}_sb")
        nc.gpsimd.dma_start(
            out=t,
            in_=io[src_name][:].bitcast(F32R).rearrange(
                "(j p) n -> p j n", p=P))
        w[key] = t

    big_w("wqkv", "Wqkv", DC, 3 * D)
    big_w("w1", "W1", DC, 2 * D)
    big_w("w2", "W2", D2C, D)
    big_w("wo", "Wo", DC, D)

    for nm, width in (("bqkv", 3 * D), ("bo", D), ("b1", 2 * D), ("b2", D)):
        t = singles.tile([1, width], F32, name=f"row_{nm}")
        nc.gpsimd.dma_start(out=t, in_=io[nm][None, :])
        w[nm] = t

    for nm in ("g1", "be1", "g2", "be2"):
        t = singles.tile([P, D], F32, name=f"bc_{nm}")
        nc.gpsimd.dma_start(out=t, in_=io[nm][None, :].to_broadcast((P, D)))
        w[nm] = t

    # One kernel-long accumulation group of 1x1 "gate" matmuls, used to make
    # PE observe the (8-lane round-robin) weight DMA completion semaphores
    # on cheap instructions before the first weight use.  The same PSUM bank
    # hosts the warm-keeper accumulator in a disjoint column strip.
    N_GATES = 8
    dw_ps = psD.tile([P, 2 * P], F32, tag="dw", name="dw_ps")
    _gate = {"i": 0}

    def gate_mm(el):
        nc.tensor.matmul(dw_ps[0:1, 0:1], lhsT=el, rhs=el,
                         start=(_gate["i"] == 0),
                         stop=(_gate["i"] == N_GATES - 1))
        _gate["i"] += 1

    def observe_weight_dmas():
        for nm in ("wqkv", "w1", "w2", "wo"):
            gate_mm(w[nm][0:1, 0, 0:1].bitcast(F32))
        for nm in ("bqkv", "bo", "b1", "b2"):
            gate_mm(w[nm][0:1, 0:1])

    # HAM warm-keeper: N=128 fp32 matmuls (4-pass, ~213ns each at full
    # clock), sprinkled through PE-idle stretches so the HAM activity
    # monitor keeps the clock at 2.4 GHz (PE transposes and waits don't
    # register as "busy", so K drops to 4/8 otherwise).  Each tick READS
    # the caller-supplied `anchor` tile: the Tile scheduler topologically
    # reorders each engine's queue, so a dependency-free matmul would be
    # hoisted to the very front of the PE stream instead of staying in
    # its block (values are irrelevant — the strip is never read).
    _warm = {"open": False, "n": 0}

    def warm_tick(k, anchor):
        a = anchor if anchor.dtype == F32 else anchor.bitcast(F32)
        for _ in range(k):
            nc.tensor.matmul(dw_ps[:, P:2 * P], lhsT=a, rhs=a,
                             start=(not _warm["open"]), stop=False)
            _warm["open"] = True
            _warm["n"] += 1

    def warm_finish():
        if _warm["open"]:
            nc.tensor.matmul(dw_ps[:, P:2 * P], lhsT=warm_sb, rhs=warm_sb,
                             start=False, stop=True)
            _warm["open"] = False

    def bias_mm(psum_ap, bias_row_ap):
        """Final accumulation-group matmul adding a [1, N] bias row to all
        output rows: psum += ones[K=1, M=P].T @ bias[K=1, N]."""
        nc.tensor.matmul(psum_ap, lhsT=ones_row,
                         rhs=bias_row_ap, start=False, stop=True)

    def transpose_chunks(src, nch, tag, dtype=F32):
        """[P, nch*P] SBUF -> [P, nch, P] SBUF holding src^T chunks."""
        dst = work1.tile([P, nch, P], dtype, tag=tag)
        for j in range(nch):
            ps = psT.tile([P, P], F32, tag="pst")
            nc.tensor.transpose(ps, src[:, j * P:(j + 1) * P], ident)
            nc.vector.tensor_copy(dst[:, j], ps)
        return dst

    def layernorm(src, g_b, be_b, tag):
        stats = work1.tile([P, 6], F32, tag=tag + "_st")
        nc.vector.bn_stats(out=stats, in_=src)
        mv = work1.tile([P, 2], F32, tag=tag + "_mv")
        nc.vector.bn_aggr(out=mv, in_=stats)
        rstd = work1.tile([P, 1], F32, tag=tag + "_rs")
        nc.scalar.activation(rstd, mv[:, 1:2], ACTF.Sqrt, bias=eps_t)
        nc.vector.reciprocal(rstd, rstd)
        out_t = work1.tile([P, D], F32, tag=tag)
        nc.vector.tensor_scalar(out_t, src, scalar1=mv[:, 0:1], scalar2=rstd,
                                op0=OP.subtract, op1=OP.mult)
        nc.vector.tensor_mul(out_t, out_t, g_b)
        nc.vector.tensor_add(out_t, out_t, be_b)
        return out_t

    def mask_prep(b):
        """Returns (ms_t [P,L] row-normalized mask, factor [P,P])."""
        mb = mb_t[b]
        total = work1.tile([P, 1], F32, tag="total")
        nc.vector.reduce_sum(total, mb, axis=AX.X, op=OP.add)
        rden = work1.tile([P, 1], F32, tag="rden")
        nc.vector.tensor_scalar_max(rden, total, 1.0)
        nc.vector.reciprocal(rden, rden)
        ms_t = work1.tile([P, L], F32, tag=f"ms{b}", name=f"ms{b}")
        nc.vector.tensor_scalar_mul(ms_t, mb, rden)

        mT = transpose_chunks(mb, LC, "mT")      # raw mask^T  [l, c]

        # joint[c,e] = sum_l m[c,l] m[e,l]
        joint_ps = psMM.tile([P, P], F32, tag="mm")
        for ch in range(LC):
            nc.tensor.matmul(joint_ps, lhsT=mT[:, ch], rhs=mT[:, ch],
                             start=(ch == 0), stop=(ch == LC - 1))
        # broadcast total^T along rows
        totT_ps = psT.tile([1, P], F32, tag="pst")
        nc.tensor.transpose(totT_ps, total, ident)
        tot_row = work1.tile([1, P], F32, tag="totrow")
        nc.vector.tensor_copy(tot_row, totT_ps)
        totb_ps = psT.tile([P, P], F32, tag="pst")
        nc.tensor.matmul(totb_ps, lhsT=ones_row, rhs=tot_row,
                         start=True, stop=True)
        # factor = (0.5 + joint / max(total[c]+total[e], 1)) / sqrt(D)
        factor = work1.tile([P, P], F32, tag=f"factor{b}", name=f"factor{b}")
        nc.vector.tensor_scalar_add(factor, totb_ps, total)
        nc.vector.tensor_scalar_max(factor, factor, 1.0)
        nc.vector.reciprocal(factor, factor)
        nc.vector.tensor_mul(factor, factor, joint_ps)
        nc.vector.tensor_scalar(factor, factor, 0.5, 1.0 / math.sqrt(D),
                                op0=OP.add, op1=OP.mult)
        return ms_t, factor

    def pooling(b, ms_t, inject=None, warm_k=0):
        """Masked-mean pooling -> E_sb [P(c), D].

        E += diag(ms[:, l]) @ x[b, :, l, :], accumulated over l in PSUM.
        inject: optional {block_idx: [fn, ...]} extra issue hooks (used to
        interleave the previous batch's transformer stages into the
        stream).
        """
        psum_E = psE.tile([P, D], F32, tag="psE")
        for ib, l0 in enumerate(range(0, L, NLB)):
            xt = xpool.tile([P, NLB, D], F32R, tag="xt")
            eng = nc.sync if ib % 2 == 0 else nc.scalar
            eng.dma_start(out=xt, in_=x[b, :, l0:l0 + NLB, :].bitcast(F32R))
            if inject and ib in inject:
                for fn in inject[ib]:
                    fn()
            diag = work2.tile([P, NLB, P], F32R, tag="diag")
            nc.vector.tensor_tensor(
                diag, identN,
                ms_t[:, l0:l0 + NLB, None].to_broadcast((P, NLB, P)),
                OP.mult)
            for i in range(NLB):
                nc.tensor.matmul(
                    psum_E,
                    lhsT=diag[:, i],
                    rhs=xt[:, i],
                    start=(l0 == 0 and i == 0),
                    stop=(l0 == L - NLB and i == NLB - 1),
                )
            if warm_k:
                warm_tick(warm_k, xt[:, 0, 0:P])
        E_sb = work2.tile([P, D], F32, tag="E")
        nc.vector.tensor_copy(E_sb, psum_E)
        return E_sb

    def transformer_stages(b, E_box, factor, warm=False):
        """Returns a list of stage closures computing y[b] from E_box[0].

        warm=True pads the PE-idle gaps after DVE/ACT-heavy stages with
        warm-keeper matmuls (used for the serial tail batch).
        """
        st = {}

        def wt(k, anchor):
            if warm:
                warm_tick(k, anchor)

        def s_et():
            st["ET"] = transpose_chunks(E_box[0], DC, "ET", F32R)

        def mk_qkv(i, name, dtype):
            def go():
                ps = psMM.tile([P, D], F32, tag="mm", name=f"qkv_ps{i}")
                for j in range(DC):
                    nc.tensor.matmul(ps, lhsT=st["ET"][:, j],
                                     rhs=w["wqkv"][:, j, i * D:(i + 1) * D],
                                     start=(j == 0), stop=False)
                bias_mm(ps, w["bqkv"][:, i * D:(i + 1) * D])
                t = work1.tile([P, D], dtype, tag=name, name=name)
                nc.vector.tensor_copy(t, ps)
                st[name] = t
            return go

        def s_qkT():
            st["qT"] = transpose_chunks(st["q"], DC, "qT")
            st["kT"] = transpose_chunks(st["k"], DC, "kT")

        def s_S():
            S_ps = psMM.tile([P, P], F32, tag="mm")
            for j in range(DC):
                nc.tensor.matmul(S_ps, lhsT=st["qT"][:, j],
                                 rhs=st["kT"][:, j],
                                 start=(j == 0), stop=(j == DC - 1))
            Sb = work1.tile([P, P], F32, tag="Sb")
            nc.vector.tensor_mul(Sb, S_ps, factor)
            nmax = work1.tile([P, 1], F32, tag="nmax")
            nc.vector.reduce_max(nmax, Sb, axis=AX.X, negate=True)
            st["Sb"], st["nmax"] = Sb, nmax
            wt(3, Sb)

        def s_soft():
            Pexp = work1.tile([P, P], F32, tag="Pexp")
            sumexp = work1.tile([P, 1], F32, tag="sumexp")
            nc.scalar.activation(Pexp, st["Sb"], ACTF.Exp, bias=st["nmax"],
                                 scale=1.0, accum_out=sumexp)
            rinv = work1.tile([P, 1], F32, tag="rinv")
            nc.vector.reciprocal(rinv, sumexp)
            # normalize rows of exp(S) so attn = A @ v directly
            nc.vector.tensor_scalar_mul(Pexp, Pexp, rinv)
            st["Pexp"] = Pexp
            wt(4, Pexp)

        def s_PT():
            PT_ps = psT.tile([P, P], F32, tag="pst")
            nc.tensor.transpose(PT_ps, st["Pexp"], ident)
            PT_sb = work1.tile([P, P], F32R, tag="PT")
            nc.vector.tensor_copy(PT_sb, PT_ps)
            st["PT"] = PT_sb
            wt(2, PT_sb)

        def s_attn():
            attn_ps = psMM.tile([P, D], F32, tag="mm")
            nc.tensor.matmul(attn_ps, lhsT=st["PT"], rhs=st["v"],
                             start=True, stop=True)
            attn_sb = work1.tile([P, D], F32, tag="attnsb")
            nc.vector.tensor_copy(attn_sb, attn_ps)
            st["attn"] = attn_sb
            wt(2, attn_sb[:, 0:P])

        def s_attnT():
            st["attnT"] = transpose_chunks(st["attn"], DC, "attnT", F32R)

        def s_o_ln1():
            o_ps = psMM.tile([P, D], F32, tag="mm")
            for j in range(DC):
                nc.tensor.matmul(o_ps, lhsT=st["attnT"][:, j],
                                 rhs=w["wo"][:, j],
                                 start=(j == 0), stop=False)
            bias_mm(o_ps, w["bo"])
            res1 = work1.tile([P, D], F32, tag="res1")
            nc.vector.tensor_add(res1, o_ps, E_box[0])
            st["E1"] = layernorm(res1, w["g1"], w["be1"], "E1")
            wt(5, st["E1"][:, 0:P])

        def s_E1T():
            st["E1T"] = transpose_chunks(st["E1"], DC, "E1T", F32R)

        def mk_h(i):
            def go():
                ps = psMM.tile([P, D], F32, tag="mm", name=f"h_ps{i}")
                for j in range(DC):
                    nc.tensor.matmul(ps, lhsT=st["E1T"][:, j],
                                     rhs=w["w1"][:, j, i * D:(i + 1) * D],
                                     start=(j == 0), stop=False)
                bias_mm(ps, w["b1"][:, i * D:(i + 1) * D])
                st[f"h{i}"] = ps
            return go

        def mk_gelu(i):
            def go():
                # reuse a dead [P, D] slot (res1 after LN1 / attnsb after
                # its transposes) for the gelu input copy
                hx = work1.tile([P, D], F32, tag="res1" if i == 0 else "attnsb",
                                name=f"hx{i}")
                nc.vector.tensor_copy(hx, st[f"h{i}"])
                g = work1.tile([P, D], F32, tag=f"g{i}", name=f"g{i}")
                nc.vector.tensor_mul(g, hx, hx)
                nc.vector.tensor_scalar(g, g, 0.044715, 1.0,
                                        op0=OP.mult, op1=OP.add)
                nc.vector.tensor_mul(g, g, hx)
                nc.scalar.activation(g, g, ACTF.Tanh,
                                     scale=math.sqrt(2.0 / math.pi))
                nc.vector.tensor_scalar(g, g, 1.0, 0.5,
                                        op0=OP.add, op1=OP.mult)
                nc.vector.tensor_mul(g, g, hx)
                st[f"g{i}"] = g
                wt(4, g[:, 0:P])
            return go

        def s_hT():
            dst = work1.tile([P, D2C, P], F32R, tag="hT")
            for half in range(2):
                src = st[f"g{half}"]
                for j in range(DC):
                    ps = psT.tile([P, P], F32, tag="pst")
                    nc.tensor.transpose(ps, src[:, j * P:(j + 1) * P], ident)
                    nc.vector.tensor_copy(dst[:, half * DC + j], ps)
            st["hT"] = dst

        def s_out():
            o2_ps = psMM.tile([P, D], F32, tag="mm")
            for jj in range(D2C):
                nc.tensor.matmul(o2_ps, lhsT=st["hT"][:, jj],
                                 rhs=w["w2"][:, jj],
                                 start=(jj == 0), stop=False)
            bias_mm(o2_ps, w["b2"])
            res2 = work1.tile([P, D], F32, tag="res2")
            nc.vector.tensor_add(res2, o2_ps, st["E1"])
            y_sb = layernorm(res2, w["g2"], w["be2"], "yln")
            nc.gpsimd.dma_start(out=y[b], in_=y_sb)

        return [s_et,
                mk_qkv(0, "q", F32), mk_qkv(1, "k", F32), mk_qkv(2, "v", F32R),
                s_qkT, s_S, s_soft, s_PT, s_attn, s_attnT, s_o_ln1,
                s_E1T, mk_h(0), mk_h(1), mk_gelu(0), mk_gelu(1),
                s_hT, s_out]

    # ---- main schedule ----------------------------------------------------
    ms0, factor0 = mask_prep(0)
    ms1, factor1 = mask_prep(1)

    E_box = [None]
    # warm_k=1: one ~213ns throwaway matmul per 2.56us block keeps the HAM
    # activity monitor from halving the clock mid-stream (at half clock the
    # PE's margin over the DMA delivery rate is too thin to stay ahead).
    E_box[0] = pooling(0, ms0, warm_k=1)
    observe_weight_dmas()

    # transformer(0) stages ride between pooling(1) blocks so the x stream
    # never stalls on a busy PE.
    stages0 = transformer_stages(0, E_box, factor0)
    inject = {1 + 2 * i: [fn] for i, fn in enumerate(stages0)}
    E1_sb = pooling(1, ms1, inject=inject, warm_k=1)

    E_box[0] = E1_sb
    for fn in transformer_stages(1, E_box, factor1, warm=True):
        fn()
    warm_finish()


def build_module() -> bass.Bass:
    # HWDGE DMA completion semaphores: keep the framework's default 8-lane
    # round-robin.  The framework serializes same-lane DMAs (each issue
    # waits for the previous same-lane completion, keeping the cumulative
    # counter unambiguous), so pinning one lane per ring — as an earlier
    # version did to satisfy the walrus one-sync-wait-per-instruction limit
    # — caps every ring at ONE outstanding transfer (~250 GB/s/ring with
    # the ~1.5us issue+semaphore latency exposed per tile).  With 8 lanes
    # the rings run ~4 transfers deep and the latency pipelines away; any
    # instruction that ends up needing several lane waits is handled by the
    # NoOp spill pass below.
    nc = bass.Bass()
    io = {}
    io["x"] = nc.declare_dram_parameter("x", [B_LOCAL, C, L, D], F32,
                                        isOutput=False)
    io["mask"] = nc.declare_dram_parameter("mask", [B_LOCAL, C, L], F32,
                                           isOutput=False)
    shapes = {
        "Wqkv": [D, 3 * D], "bqkv": [3 * D], "Wo": [D, D], "bo": [D],
        "W1": [D, 2 * D], "b1": [2 * D], "W2": [2 * D, D], "b2": [D],
        "g1": [D], "be1": [D], "g2": [D], "be2": [D],
    }
    for nm in WEIGHT_NAMES:
        io[nm] = nc.declare_dram_parameter(nm, shapes[nm], F32, isOutput=False)
    io["y"] = nc.declare_dram_parameter("y", [B_LOCAL, C, D], F32,
                                        isOutput=True)

    with tile.TileContext(nc) as tc:
        with ExitStack() as ctx:
            _build_kernel_body(ctx, tc, io)
    _split_multi_waits(nc)
    return nc


def _split_multi_waits(nc: bass.Bass) -> int:
    """The walrus codegen in this toolchain accepts at most ONE sync-wait
    command per ISA instruction. Tile's semaphore assignment can attach
    several. Spill all but the last wait of each instruction onto NoOp
    instructions (same engine, inserted just before it), each carrying a
    single wait — execution-equivalent since the engine stream is in-order.
    """
    import bass_rust as _br
    fn = nc.m.functions[0]
    n_spilled = 0
    for blk in fn.blocks:
        out = []
        changed = False
        for inst in blk.instructions:
            si = inst.sync_info
            if si is not None and len(si.on_wait) > 1:
                waits = list(si.on_wait)
                for wv in waits[:-1]:
                    n_spilled += 1
                    nop = mybir.InstNoOp(
                        name=f"I-wspill-{n_spilled}",
                        engine=inst.engine,
                        sync_info=_br.SyncInfo(on_wait=[wv], on_update=[]),
                        bass_nofuse=True,
                    )
                    nc.register_instruction(nop)
                    out.append(nop)
                inst.sync_info = _br.SyncInfo(
                    on_wait=[waits[-1]], on_update=list(si.on_update))
                changed = True
            out.append(inst)
        if changed:
            blk.instructions = out
    return n_spilled


_NC_CACHE = None


def _get_module():
    global _NC_CACHE
    if _NC_CACHE is None:
        _NC_CACHE = build_module()
    return _NC_CACHE


def kernel(**inputs) -> np.ndarray:
    arrs = {k: np.ascontiguousarray(np.asarray(v, dtype=np.float32))
            for k, v in inputs.items()}
    nc = _get_module()
    in_maps = []
    for i in range(N_CORES):
        m = {
            "x": arrs["x"][i * B_LOCAL:(i + 1) * B_LOCAL],
            "mask": arrs["mask"][i * B_LOCAL:(i + 1) * B_LOCAL],
        }
        for nm in WEIGHT_NAMES:
            m[nm] = arrs[nm]
        in_maps.append(m)
    res = run_bass_kernel_spmd(nc, in_maps, list(range(N_CORES)))
    return np.concatenate([r["y"] for r in res.results], axis=0)


if __name__ == "__main__":
    build_module()
    print("module built OK")


# revision 14
# speedup vs baseline: 1.3968x; 1.2056x over previous
"""Trainium2 Bass kernel for ChannelHyperedgeInteraction.

Computation (per batch b):
    E  = masked-mean-pool of x over L              [C, d]
    qkv = E @ Wqkv + bqkv ; q,k,v                  [C, d] each
    S  = (q k^T / sqrt(d)) * (0.5 + 0.5*overlap)   [C, C]
    A  = softmax(S, -1) ; out = A v @ Wo + bo
    E  = LN(E + out) ; h = gelu(E W1 + b1) W2 + b2 ; return LN(E + h)

Sharding: data-parallel over B across the 8 NeuronCores (2 batches/core).
Weights are replicated. Each core computes its own y[b_local] slice; the
host concatenates.

Schedule (the dominant cost is streaming x, 134 MB/core, at the DMA
roofline ~400 GB/s):
 - x tiles alternate between the SP and ACT HWDGE rings (per-ring cap is
   ~210 GB/s; both must stay busy).  Weights/biases/broadcasts/outputs
   ride the Pool ring exclusively so they never head-of-line block an
   x tile (the previous version injected weight DMAs into the SP ring
   mid-stream, which serialized both rings for ~90 us).
 - Both batches' mask prep happens up front; transformer(0) is split
   into stages injected between pooling(1) blocks, so the PE keeps
   consuming x tiles (freeing DMA ring buffers) while batch 0's
   attention/FFN runs.  Only transformer(1) is a serial tail.
 - The masked pooling is done on the TensorEngine as
   E += diag(mask_scaled[:, l]) @ x[b, :, l, :] accumulated in PSUM;
   fp32r keeps the moving operand at 1 column/cycle.
 - HAM (the PE activity clock monitor) halves the core clock when PE
   looks idle; during the DMA-bound stream that is harmless (PE keeps
   up with the rings even at half clock), but the serial tail would run
   2x slow.  Throwaway "warm" matmuls pad the PE's idle gaps through
   the last pooling blocks and the tail DVE/ACT stretches to hold the
   clock at 2.4 GHz.  They accumulate into an unused strip of the gate
   PSUM bank.
 - The walrus codegen accepts at most ONE sync wait per ISA
   instruction: HWDGE DMA completions are pinned one semaphore lane PER
   RING (SP=0, ACT=1, Pool=2; rings complete FIFO so cumulative waits
   are safe), 1x1 "gate" matmuls make PE observe DMA semaphores on
   cheap instructions, and a post-pass spills any remaining multi-waits
   onto same-engine NoOps.
"""

import math
from contextlib import ExitStack

import numpy as np

import concourse.bass as bass
import concourse.mybir as mybir
import concourse.tile as tile
from concourse.bass_utils import run_bass_kernel_spmd
from concourse.masks import make_identity

F32 = mybir.dt.float32
F32R = mybir.dt.float32r
AX = mybir.AxisListType
OP = mybir.AluOpType
ACTF = mybir.ActivationFunctionType

B, C, L, D = 16, 128, 256, 512
N_CORES = 8
B_LOCAL = B // N_CORES  # 2
P = 128
LC = L // P   # 2 l-chunks
DC = D // P   # 4 d-chunks
D2C = (2 * D) // P  # 8 chunks of the FFN hidden dim
NLB = 4  # l-positions per x DMA (1 MB transfers)
NBLK = L // NLB

WEIGHT_NAMES = ("Wqkv", "bqkv", "Wo", "bo", "W1", "b1", "W2", "b2",
                "g1", "be1", "g2", "be2")


def _build_kernel_body(ctx: ExitStack, tc: "tile.TileContext", io: dict):
    nc = tc.nc
    x, mask, y = io["x"], io["mask"], io["y"]

    singles = ctx.enter_context(tc.tile_pool(name="singles", bufs=1))
    xpool = ctx.enter_context(tc.tile_pool(name="xpool", bufs=8))
    work2 = ctx.enter_context(tc.tile_pool(name="work2", bufs=2))
    work1 = ctx.enter_context(tc.tile_pool(name="work1", bufs=1))
    psE = ctx.enter_context(tc.tile_pool(name="psE", bufs=1, space="PSUM"))
    psT = ctx.enter_context(tc.tile_pool(name="psT", bufs=2, space="PSUM"))
    psMM = ctx.enter_context(tc.tile_pool(name="psMM", bufs=4, space="PSUM"))
    psD = ctx.enter_context(tc.tile_pool(name="psD", bufs=1, space="PSUM"))

    ones_row = singles.tile([1, P], F32)
    nc.vector.memset(ones_row, 1.0)
    eps_t = singles.tile([P, 1], F32)
    nc.vector.memset(eps_t, 1e-5)
    # scratch operand for HAM warm-keeper matmuls (content irrelevant)
    warm_sb = singles.tile([P, P], F32)
    nc.vector.memset(warm_sb, 1.0)
    # fp32 identity: the ONLY gpsimd-built constant, created first so the
    # PE can start consuming x within ~2us of kernel start (if the PE ever
    # falls behind the rings, the dma_start issues begin blocking on PE
    # progress and the stream degenerates into a few-tiles-in-flight limit
    # cycle at ~70% throughput).  The warmup transpose makes PE observe the
    # gpsimd semaphore at its latest tick, so no later PE instruction needs
    # a Pool wait.
    ident = singles.tile([P, P], F32)
    make_identity(nc, ident)
    wu_ps = psT.tile([P, P], F32, tag="pst")
    nc.tensor.transpose(wu_ps, ident, ident)
    # identity blocks (template for the pooling diag weights): replicated
    # from `ident` on the DVE, which is much faster than 4 gpsimd
    # affine-selects.
    identN = singles.tile([P, NLB, P], F32)
    for i in range(NLB):
        nc.vector.tensor_copy(identN[:, i], ident)

    # --- masks: both batches' DMAs first in the SP ring ---------------------
    mb_t = {}
    for b in range(B_LOCAL):
        t = work1.tile([P, L], F32, tag=f"mb{b}", name=f"mb{b}")
        nc.sync.dma_start(out=t, in_=mask[b])
        mb_t[b] = t

    # --- weights: all on the Pool ring, issued up front ---------------------
    # (their completions are observed via gate matmuls after pooling(0))
    w = {}

    def big_w(key, src_name, nch, width):
        t = singles.tile([P, nch, width], F32R, name=f"{key}_sb")
        nc.gpsimd.dma_start(
            out=t,
            in_=io[src_name][:].bitcast(F32R).rearrange(
                "(j p) n -> p j n", p=P))
        w[key] = t

    big_w("wqkv", "Wqkv", DC, 3 * D)
    big_w("w1", "W1", DC, 2 * D)
    big_w("w2", "W2", D2C, D)
    big_w("wo", "Wo", DC, D)

    for nm, width in (("bqkv", 3 * D), ("bo", D), ("b1", 2 * D), ("b2", D)):
        t = singles.tile([1, width], F32, name=f"row_{nm}")
        nc.gpsimd.dma_start(out=t, in_=io[nm][None, :])
        w[nm] = t

    for nm in ("g1", "be1", "g2", "be2"):
        t = singles.tile([P, D], F32, name=f"bc_{nm}")
        nc.gpsimd.dma_start(out=t, in_=io[nm][None, :].to_broadcast((P, D)))
        w[nm] = t

    # One kernel-long accumulation group of 1x1 "gate" matmuls, used to make
    # PE observe the (8-lane round-robin) weight DMA completion semaphores
    # on cheap instructions before the first weight use.  The same PSUM bank
    # hosts the warm-keeper accumulator in a disjoint column strip.
    N_GATES = 8
    dw_ps = psD.tile([P, 2 * P], F32, tag="dw", name="dw_ps")
    _gate = {"i": 0}

    def gate_mm(el):
        nc.tensor.matmul(dw_ps[0:1, 0:1], lhsT=el, rhs=el,
                         start=(_gate["i"] == 0),
                         stop=(_gate["i"] == N_GATES - 1))
        _gate["i"] += 1

    def observe_weight_dmas():
        for nm in ("wqkv", "w1", "w2", "wo"):
            gate_mm(w[nm][0:1, 0, 0:1].bitcast(F32))
        for nm in ("bqkv", "bo", "b1", "b2"):
            gate_mm(w[nm][0:1, 0:1])

    # HAM warm-keeper: N=128 fp32 matmuls (4-pass, ~213ns each at full
    # clock), sprinkled through PE-idle stretches so the HAM activity
    # monitor keeps the clock at 2.4 GHz (PE transposes and waits don't
    # register as "busy", so K drops to 4/8 otherwise).  Each tick READS
    # the caller-supplied `anchor` tile: the Tile scheduler topologically
    # reorders each engine's queue, so a dependency-free matmul would be
    # hoisted to the very front of the PE stream instead of staying in
    # its block (values are irrelevant — the strip is never read).
    _warm = {"open": False, "n": 0}

    def warm_tick(k, anchor):
        a = anchor if anchor.dtype == F32 else anchor.bitcast(F32)
        for _ in range(k):
            nc.tensor.matmul(dw_ps[:, P:2 * P], lhsT=a, rhs=a,
                             start=(not _warm["open"]), stop=False)
            _warm["open"] = True
            _warm["n"] += 1

    def warm_finish():
        if _warm["open"]:
            nc.tensor.matmul(dw_ps[:, P:2 * P], lhsT=warm_sb, rhs=warm_sb,
                             start=False, stop=True)
            _warm["open"] = False

    def bias_mm(psum_ap, bias_row_ap):
        """Final accumulation-group matmul adding a [1, N] bias row to all
        output rows: psum += ones[K=1, M=P].T @ bias[K=1, N]."""
        nc.tensor.matmul(psum_ap, lhsT=ones_row,
                         rhs=bias_row_ap, start=False, stop=True)

    def transpose_chunks(src, nch, tag, dtype=F32):
        """[P, nch*P] SBUF -> [P, nch, P] SBUF holding src^T chunks."""
        dst = work1.tile([P, nch, P], dtype, tag=tag)
        for j in range(nch):
            ps = psT.tile([P, P], F32, tag="pst")
            nc.tensor.transpose(ps, src[:, j * P:(j + 1) * P], ident)
            nc.vector.tensor_copy(dst[:, j], ps)
        return dst

    def layernorm(src, g_b, be_b, tag):
        stats = work1.tile([P, 6], F32, tag=tag + "_st")
        nc.vector.bn_stats(out=stats, in_=src)
        mv = work1.tile([P, 2], F32, tag=tag + "_mv")
        nc.vector.bn_aggr(out=mv, in_=stats)
        rstd = work1.tile([P, 1], F32, tag=tag + "_rs")
        nc.scalar.activation(rstd, mv[:, 1:2], ACTF.Sqrt, bias=eps_t)
        nc.vector.reciprocal(rstd, rstd)
        out_t = work1.tile([P, D], F32, tag=tag)
        nc.vector.tensor_scalar(out_t, src, scalar1=mv[:, 0:1], scalar2=rstd,
                                op0=OP.subtract, op1=OP.mult)
        nc.vector.tensor_mul(out_t, out_t, g_b)
        nc.vector.tensor_add(out_t, out_t, be_b)
        return out_t

    def mask_prep(b):
        """Returns (ms_t [P,L] row-normalized mask, factor [P,P])."""
        mb = mb_t[b]
        total = work1.tile([P, 1], F32, tag="total")
        nc.vector.reduce_sum(total, mb, axis=AX.X, op=OP.add)
        rden = work1.tile([P, 1], F32, tag="rden")
        nc.vector.tensor_scalar_max(rden, total, 1.0)
        nc.vector.reciprocal(rden, rden)
        ms_t = work1.tile([P, L], F32, tag=f"ms{b}", name=f"ms{b}")
        nc.vector.tensor_scalar_mul(ms_t, mb, rden)

        mT = transpose_chunks(mb, LC, "mT")      # raw mask^T  [l, c]

        # joint[c,e] = sum_l m[c,l] m[e,l]
        joint_ps = psMM.tile([P, P], F32, tag="mm")
        for ch in range(LC):
            nc.tensor.matmul(joint_ps, lhsT=mT[:, ch], rhs=mT[:, ch],
                             start=(ch == 0), stop=(ch == LC - 1))
        # broadcast total^T along rows
        totT_ps = psT.tile([1, P], F32, tag="pst")
        nc.tensor.transpose(totT_ps, total, ident)
        tot_row = work1.tile([1, P], F32, tag="totrow")
        nc.vector.tensor_copy(tot_row, totT_ps)
        totb_ps = psT.tile([P, P], F32, tag="pst")
        nc.tensor.matmul(totb_ps, lhsT=ones_row, rhs=tot_row,
                         start=True, stop=True)
        # factor = (0.5 + joint / max(total[c]+total[e], 1)) / sqrt(D)
        factor = work1.tile([P, P], F32, tag=f"factor{b}", name=f"factor{b}")
        nc.vector.tensor_scalar_add(factor, totb_ps, total)
        nc.vector.tensor_scalar_max(factor, factor, 1.0)
        nc.vector.reciprocal(factor, factor)
        nc.vector.tensor_mul(factor, factor, joint_ps)
        nc.vector.tensor_scalar(factor, factor, 0.5, 1.0 / math.sqrt(D),
                                op0=OP.add, op1=OP.mult)
        return ms_t, factor

    def pooling(b, ms_t, inject=None, warm_k=0):
        """Masked-mean pooling -> E_sb [P(c), D].

        E += diag(ms[:, l]) @ x[b, :, l, :], accumulated over l in PSUM.
        inject: optional {block_idx: [fn, ...]} extra issue hooks (used to
        interleave the previous batch's transformer stages into the
        stream).
        """
        psum_E = psE.tile([P, D], F32, tag="psE")
        for ib, l0 in enumerate(range(0, L, NLB)):
            xt = xpool.tile([P, NLB, D], F32R, tag="xt")
            eng = nc.sync if ib % 2 == 0 else nc.scalar
            eng.dma_start(out=xt, in_=x[b, :, l0:l0 + NLB, :].bitcast(F32R))
            if inject and ib in inject:
                for fn in inject[ib]:
                    fn()
            diag = work2.tile([P, NLB, P], F32R, tag="diag")
            nc.vector.tensor_tensor(
                diag, identN,
                ms_t[:, l0:l0 + NLB, None].to_broadcast((P, NLB, P)),
                OP.mult)
            for i in range(NLB):
                nc.tensor.matmul(
                    psum_E,
                    lhsT=diag[:, i],
                    rhs=xt[:, i],
                    start=(l0 == 0 and i == 0),
                    stop=(l0 == L - NLB and i == NLB - 1),
                )
            if warm_k:
                warm_tick(warm_k, xt[:, 0, 0:P])
        E_sb = work2.tile([P, D], F32, tag="E")
        nc.vector.tensor_copy(E_sb, psum_E)
        return E_sb

    def transformer_stages(b, E_box, factor, warm=False):
        """Returns a list of stage closures computing y[b] from E_box[0].

        warm=True pads the PE-idle gaps after DVE/ACT-heavy stages with
        warm-keeper matmuls (used for the serial tail batch).
        """
        st = {}

        def wt(k, anchor):
            if warm:
                warm_tick(k, anchor)

        def s_et():
            st["ET"] = transpose_chunks(E_box[0], DC, "ET", F32R)

        def mk_qkv(i, name, dtype):
            def go():
                ps = psMM.tile([P, D], F32, tag="mm", name=f"qkv_ps{i}")
                for j in range(DC):
                    nc.tensor.matmul(ps, lhsT=st["ET"][:, j],
                                     rhs=w["wqkv"][:, j, i * D:(i + 1) * D],
                                     start=(j == 0), stop=False)
                bias_mm(ps, w["bqkv"][:, i * D:(i + 1) * D])
                t = work1.tile([P, D], dtype, tag=name, name=name)
                nc.vector.tensor_copy(t, ps)
                st[name] = t
            return go

        def s_qkT():
            st["qT"] = transpose_chunks(st["q"], DC, "qT")
            st["kT"] = transpose_chunks(st["k"], DC, "kT")

        def s_S():
            S_ps = psMM.tile([P, P], F32, tag="mm")
            for j in range(DC):
                nc.tensor.matmul(S_ps, lhsT=st["qT"][:, j],
                                 rhs=st["kT"][:, j],
                                 start=(j == 0), stop=(j == DC - 1))
            Sb = work1.tile([P, P], F32, tag="Sb")
            nc.vector.tensor_mul(Sb, S_ps, factor)
            nmax = work1.tile([P, 1], F32, tag="nmax")
            nc.vector.reduce_max(nmax, Sb, axis=AX.X, negate=True)
            st["Sb"], st["nmax"] = Sb, nmax
            wt(3, Sb)

        def s_soft():
            Pexp = work1.tile([P, P], F32, tag="Pexp")
            sumexp = work1.tile([P, 1], F32, tag="sumexp")
            nc.scalar.activation(Pexp, st["Sb"], ACTF.Exp, bias=st["nmax"],
                                 scale=1.0, accum_out=sumexp)
            rinv = work1.tile([P, 1], F32, tag="rinv")
            nc.vector.reciprocal(rinv, sumexp)
            # normalize rows of exp(S) so attn = A @ v directly
            nc.vector.tensor_scalar_mul(Pexp, Pexp, rinv)
            st["Pexp"] = Pexp
            wt(4, Pexp)

        def s_PT():
            PT_ps = psT.tile([P, P], F32, tag="pst")
            nc.tensor.transpose(PT_ps, st["Pexp"], ident)
            PT_sb = work1.tile([P, P], F32R, tag="PT")
            nc.vector.tensor_copy(PT_sb, PT_ps)
            st["PT"] = PT_sb
            wt(2, PT_sb)

        def s_attn():
            attn_ps = psMM.tile([P, D], F32, tag="mm")
            nc.tensor.matmul(attn_ps, lhsT=st["PT"], rhs=st["v"],
                             start=True, stop=True)
            attn_sb = work1.tile([P, D], F32, tag="attnsb")
            nc.vector.tensor_copy(attn_sb, attn_ps)
            st["attn"] = attn_sb
            wt(2, attn_sb[:, 0:P])

        def s_attnT():
            st["attnT"] = transpose_chunks(st["attn"], DC, "attnT", F32R)

        def s_o_ln1():
            o_ps = psMM.tile([P, D], F32, tag="mm")
            for j in range(DC):
                nc.tensor.matmul(o_ps, lhsT=st["attnT"][:, j],
                                 rhs=w["wo"][:, j],
                                 start=(j == 0), stop=False)
            bias_mm(o_ps, w["bo"])
            res1 = work1.tile([P, D], F32, tag="res1")
            nc.vector.tensor_add(res1, o_ps, E_box[0])
            st["E1"] = layernorm(res1, w["g1"], w["be1"], "E1")
            wt(5, st["E1"][:, 0:P])

        def s_E1T():
            st["E1T"] = transpose_chunks(st["E1"], DC, "E1T", F32R)

        def mk_h(i):
            def go():
                ps = psMM.tile([P, D], F32, tag="mm", name=f"h_ps{i}")
                for j in range(DC):
                    nc.tensor.matmul(ps, lhsT=st["E1T"][:, j],
                                     rhs=w["w1"][:, j, i * D:(i + 1) * D],
                                     start=(j == 0), stop=False)
                bias_mm(ps, w["b1"][:, i * D:(i + 1) * D])
                st[f"h{i}"] = ps
            return go

        def mk_gelu(i):
            def go():
                # reuse a dead [P, D] slot (res1 after LN1 / attnsb after
                # its transposes) for the gelu input copy
                hx = work1.tile([P, D], F32, tag="res1" if i == 0 else "attnsb",
                                name=f"hx{i}")
                nc.vector.tensor_copy(hx, st[f"h{i}"])
                g = work1.tile([P, D], F32, tag=f"g{i}", name=f"g{i}")
                nc.vector.tensor_mul(g, hx, hx)
                nc.vector.tensor_scalar(g, g, 0.044715, 1.0,
                                        op0=OP.mult, op1=OP.add)
                nc.vector.tensor_mul(g, g, hx)
                nc.scalar.activation(g, g, ACTF.Tanh,
                                     scale=math.sqrt(2.0 / math.pi))
                nc.vector.tensor_scalar(g, g, 1.0, 0.5,
                                        op0=OP.add, op1=OP.mult)
                nc.vector.tensor_mul(g, g, hx)
                st[f"g{i}"] = g
                wt(4, g[:, 0:P])
            return go

        def s_hT():
            dst = work1.tile([P, D2C, P], F32R, tag="hT")
            for half in range(2):
                src = st[f"g{half}"]
                for j in range(DC):
                    ps = psT.tile([P, P], F32, tag="pst")
                    nc.tensor.transpose(ps, src[:, j * P:(j + 1) * P], ident)
                    nc.vector.tensor_copy(dst[:, half * DC + j], ps)
            st["hT"] = dst

        def s_out():
            o2_ps = psMM.tile([P, D], F32, tag="mm")
            for jj in range(D2C):
                nc.tensor.matmul(o2_ps, lhsT=st["hT"][:, jj],
                                 rhs=w["w2"][:, jj],
                                 start=(jj == 0), stop=False)
            bias_mm(o2_ps, w["b2"])
            res2 = work1.tile([P, D], F32, tag="res2")
            nc.vector.tensor_add(res2, o2_ps, st["E1"])
            y_sb = layernorm(res2, w["g2"], w["be2"], "yln")
            nc.gpsimd.dma_start(out=y[b], in_=y_sb)

        return [s_et,
                mk_qkv(0, "q", F32), mk_qkv(1, "k", F32), mk_qkv(2, "v", F32R),
                s_qkT, s_S, s_soft, s_PT, s_attn, s_attnT, s_o_ln1,
                s_E1T, mk_h(0), mk_h(1), mk_gelu(0), mk_gelu(1),
                s_hT, s_out]

    # ---- main schedule ----------------------------------------------------
    ms0, factor0 = mask_prep(0)
    ms1, factor1 = mask_prep(1)

    E_box = [None]
    # warm_k=1: one ~213ns throwaway matmul per 2.56us block keeps the HAM
    # activity monitor from halving the clock mid-stream (at half clock the
    # PE's margin over the DMA delivery rate is too thin to stay ahead).
    E_box[0] = pooling(0, ms0)
    observe_weight_dmas()

    # transformer(0) stages ride between pooling(1) blocks so the x stream
    # never stalls on a busy PE.
    stages0 = transformer_stages(0, E_box, factor0)
    inject = {1 + 2 * i: [fn] for i, fn in enumerate(stages0)}
    E1_sb = pooling(1, ms1, inject=inject)

    E_box[0] = E1_sb
    for fn in transformer_stages(1, E_box, factor1, warm=False):
        fn()
    warm_finish()


def build_module() -> bass.Bass:
    # HWDGE DMA completion semaphores: keep the framework's default 8-lane
    # round-robin.  The framework serializes same-lane DMAs (each issue
    # waits for the previous same-lane completion, keeping the cumulative
    # counter unambiguous), so pinning one lane per ring — as an earlier
    # version did to satisfy the walrus one-sync-wait-per-instruction limit
    # — caps every ring at ONE outstanding transfer (~250 GB/s/ring with
    # the ~1.5us issue+semaphore latency exposed per tile).  With 8 lanes
    # the rings run ~4 transfers deep and the latency pipelines away; any
    # instruction that ends up needing several lane waits is handled by the
    # NoOp spill pass below.
    nc = bass.Bass()
    io = {}
    io["x"] = nc.declare_dram_parameter("x", [B_LOCAL, C, L, D], F32,
                                        isOutput=False)
    io["mask"] = nc.declare_dram_parameter("mask", [B_LOCAL, C, L], F32,
                                           isOutput=False)
    shapes = {
        "Wqkv": [D, 3 * D], "bqkv": [3 * D], "Wo": [D, D], "bo": [D],
        "W1": [D, 2 * D], "b1": [2 * D], "W2": [2 * D, D], "b2": [D],
        "g1": [D], "be1": [D], "g2": [D], "be2": [D],
    }
    for nm in WEIGHT_NAMES:
        io[nm] = nc.declare_dram_parameter(nm, shapes[nm], F32, isOutput=False)
    io["y"] = nc.declare_dram_parameter("y", [B_LOCAL, C, D], F32,
                                        isOutput=True)

    with tile.TileContext(nc) as tc:
        with ExitStack() as ctx:
            _build_kernel_body(ctx, tc, io)
    _split_multi_waits(nc)
    return nc


def _split_multi_waits(nc: bass.Bass) -> int:
    """The walrus codegen in this toolchain accepts at most ONE sync-wait
    command per ISA instruction. Tile's semaphore assignment can attach
    several. Spill all but the last wait of each instruction onto NoOp
    instructions (same engine, inserted just before it), each carrying a
    single wait — execution-equivalent since the engine stream is in-order.
    """
    import bass_rust as _br
    fn = nc.m.functions[0]
    n_spilled = 0
    for blk in fn.blocks:
        out = []
        changed = False
        for inst in blk.instructions:
            si = inst.sync_info
            if si is not None and len(si.on_wait) > 1:
                waits = list(si.on_wait)
                for wv in waits[:-1]:
                    n_spilled += 1
                    nop = mybir.InstNoOp(
                        name=f"I-wspill-{n_spilled}",
                        engine=inst.engine,
                        sync_info=_br.SyncInfo(on_wait=[wv], on_update=[]),
                        bass_nofuse=True,
                    )
                    nc.register_instruction(nop)
                    out.append(nop)
                inst.sync_info = _br.SyncInfo(
                    on_wait=[waits[-1]], on_update=list(si.on_update))
                changed = True
            out.append(inst)
        if changed:
            blk.instructions = out
    return n_spilled


_NC_CACHE = None


def _get_module():
    global _NC_CACHE
    if _NC_CACHE is None:
        _NC_CACHE = build_module()
    return _NC_CACHE


def kernel(**inputs) -> np.ndarray:
    arrs = {k: np.ascontiguousarray(np.asarray(v, dtype=np.float32))
            for k, v in inputs.items()}
    nc = _get_module()
    in_maps = []
    for i in range(N_CORES):
        m = {
            "x": arrs["x"][i * B_LOCAL:(i + 1) * B_LOCAL],
            "mask": arrs["mask"][i * B_LOCAL:(i + 1) * B_LOCAL],
        }
        for nm in WEIGHT_NAMES:
            m[nm] = arrs[nm]
        in_maps.append(m)
    res = run_bass_kernel_spmd(nc, in_maps, list(range(N_CORES)))
    return np.concatenate([r["y"] for r in res.results], axis=0)


if __name__ == "__main__":
    build_module()
    print("module built OK")
